# revision 23
# baseline (speedup 1.0000x reference)
"""GAT (2-layer, PyG-style) on 8 Trainium2 NeuronCores.

Design ("degree-binned slot layout", pair-packed gather rows):
- Nodes are relabeled: ranked by in-degree, dealt round-robin to 8 cores
  (balances edges), each core's 6250 nodes sorted by degree so every
  128-node block is degree-homogeneous. Slot (core c, block b, partition p)
  = node c*6250 + 128b + p in the new labeling.
- Each dst node owns one SBUF partition slot; its in-edges lie along the
  free dim (block width W_b = max in-degree in block, ~3% padding).
  Segment-softmax + weighted aggregation become free-dim reduces: no
  per-edge scatter, no matmuls in the edge phase.
- Per-edge traffic: one dma_gather row of 512B holding the node PAIR
  (2k, 2k+1): [h_even|h_odd|a_s_even|a_s_odd]. Pair index 1+(src>>1)
  fits int16 (dma_gather requirement). A host-built parity mask zeroes
  the wrong half (and pad edges) inside the softmax weights exp(alpha).
- Softmax skips the max-subtraction (mathematically identical; alpha is
  bounded ~|6| here so exp is safe in f32).
- Layer1 -> layer2 node tables are exchanged with AllGather; everything
  runs in ONE NEFF on 8 cores (SPMD).
"""
import os
import sys
import types
import numpy as np
import ml_dtypes

# ---- problem constants (hardcoded per contest rules) ----
N = 50000
IN = 256
H1, F1 = 8, 8
HID = 64
OUT = 64
SLOPE = 0.2
EPS = 1e-16
NCORES = 8
P = 128
SHARD = N // NCORES            # 6250
NBLK = (SHARD + P - 1) // P    # 49
SHARD_PAD = NBLK * P           # 6272
NPAIR = N // 2                 # 25000
TROWS = NPAIR + 1              # +1 dummy pair row at 0
ROW = 256                      # bf16 elems per pair row (512B)
MAX_CHUNK_IDX = 8192           # gather idx per instruction (64 cols)
NEG = -1e30

_cache = {}


# ======================================================================
# host-side plan (pure index bookkeeping on edge_index)
# ======================================================================
def _build_plan(edge_index):
    src0 = edge_index[0].astype(np.int64)
    dst0 = edge_index[1].astype(np.int64)
    loops = np.arange(N, dtype=np.int64)
    src = np.concatenate([src0, loops])
    dst = np.concatenate([dst0, loops])

    indeg = np.bincount(dst, minlength=N)
    rank = np.argsort(indeg, kind="stable")          # rank r -> orig node
    r_of = np.empty(N, dtype=np.int64)
    r_of[rank] = np.arange(N)
    core_of = r_of % NCORES
    pos_of = r_of // NCORES
    new_of_orig = core_of * SHARD + pos_of
    orig_of_new = np.empty(N, dtype=np.int64)
    orig_of_new[new_of_orig] = np.arange(N)

    s = new_of_orig[src]
    d = new_of_orig[dst]

    # per-core CSR over local dst
    cores = []
    W = np.zeros(NBLK, dtype=np.int64)
    for c in range(NCORES):
        m = (d // SHARD) == c
        sc = s[m]
        dc = d[m] - c * SHARD
        order = np.argsort(dc, kind="stable")
        sc, dc = sc[order], dc[order]
        deg = np.bincount(dc, minlength=SHARD)
        off = np.zeros(SHARD + 1, dtype=np.int64)
        np.cumsum(deg, out=off[1:])
        cores.append((sc, off, deg))
        degp = np.concatenate([deg, np.zeros(SHARD_PAD - SHARD, dtype=np.int64)])
        W = np.maximum(W, degp.reshape(NBLK, P).max(axis=1))
    W = np.maximum(W, 1)

    # per-core streams
    cum = np.zeros(NBLK + 1, dtype=np.int64)
    np.cumsum(W, out=cum[1:])
    SW = int(cum[-1])                        # total cols per core
    NIDX = SW * P

    idx_streams, mask_streams = [], []
    for c in range(NCORES):
        sc, off, deg = cores[c]
        idx_blk = np.zeros((SW, P), dtype=np.int16)       # w-major stream
        msk_blk = np.zeros((SW, P, 2), dtype=ml_dtypes.bfloat16)
        for b in range(NBLK):
            wb = int(W[b])
            base = int(cum[b])
            for p in range(P):
                n = b * P + p
                if n >= SHARD:
                    continue
                es = sc[off[n]: off[n + 1]]
                k = len(es)
                idx_blk[base: base + k, p] = (es >> 1) + 1
                par = (es & 1).astype(np.int64)
                msk_blk[base + np.arange(k), p, par] = 1.0
        # wrap idx stream: i = w*128+p -> [16, NIDX/16] col-major, replicate x8
        lin_idx = idx_blk.reshape(-1)                     # [NIDX] order w-major
        wrapped = np.zeros((16, NIDX // 16), dtype=np.int16)
        ii = np.arange(NIDX)
        wrapped[ii % 16, ii // 16] = lin_idx
        idx_streams.append(np.tile(wrapped, (8, 1)))
        mask_streams.append(np.ascontiguousarray(msk_blk.transpose(1, 0, 2)))  # [P, SW, 2]

    # chunking: whole blocks, <= MAX_CHUNK_IDX idx per gather
    chunks = []          # (block_lo, block_hi, col_off, ncols)
    b0 = 0
    while b0 < NBLK:
        b1 = b0 + 1
        while b1 < NBLK and (cum[b1 + 1] - cum[b0]) * P <= MAX_CHUNK_IDX:
            b1 += 1
        chunks.append((b0, b1, int(cum[b0]), int(cum[b1] - cum[b0])))
        b0 = b1

    return {
        "new_of_orig": new_of_orig,
        "orig_of_new": orig_of_new,
        "W": W, "cum": cum, "SW": SW, "NIDX": NIDX,
        "chunks": chunks,
        "idx_streams": idx_streams,
        "mask_streams": mask_streams,
    }


# ======================================================================
# bass kernel build
# ======================================================================
def _build_nc(plan):
    import concourse.bacc as bacc
    import concourse.bass as bass
    import concourse.mybir as mybir
    import concourse.tile as tile
    from concourse.library_config import mlp
    from concourse.masks import make_identity

    f32, bf16, i16 = mybir.dt.float32, mybir.dt.bfloat16, mybir.dt.int16
    AF = mybir.ActivationFunctionType
    OP = mybir.AluOpType
    AX = mybir.AxisListType

    W = plan["W"]; cum = plan["cum"]; SW = plan["SW"]; NIDX = plan["NIDX"]
    chunks = plan["chunks"]
    MAXCOLS = max(nc_ for (_, _, _, nc_) in chunks)

    nc = bacc.Bacc("TRN2", debug=False, num_swdge_queues=4)

    xT = nc.dram_tensor("xT", [IN, SHARD_PAD], bf16, kind="ExternalInput")
    idxs = nc.dram_tensor("idxs", [P, NIDX // 16], i16, kind="ExternalInput")
    pmask = nc.dram_tensor("pmask", [P, SW * 2], bf16, kind="ExternalInput")
    w1 = nc.dram_tensor("w1", [IN, HID], f32, kind="ExternalInput")
    as1 = nc.dram_tensor("as1", [1, HID], f32, kind="ExternalInput")   # flattened [H1*F1]
    ad1 = nc.dram_tensor("ad1", [1, HID], f32, kind="ExternalInput")
    b1v = nc.dram_tensor("b1v", [1, HID], f32, kind="ExternalInput")
    w2 = nc.dram_tensor("w2", [HID, OUT], f32, kind="ExternalInput")
    as2 = nc.dram_tensor("as2", [1, OUT], f32, kind="ExternalInput")
    ad2 = nc.dram_tensor("ad2", [1, OUT], f32, kind="ExternalInput")
    b2v = nc.dram_tensor("b2v", [1, OUT], f32, kind="ExternalInput")
    drow = nc.dram_tensor("drow", [1, ROW], bf16, kind="ExternalInput")
    out = nc.dram_tensor("out", [SHARD_PAD, OUT], f32, kind="ExternalOutput")

    table1 = nc.dram_tensor("table1", [TROWS, ROW], bf16)
    table2 = nc.dram_tensor("table2", [TROWS, ROW], bf16)
    my1 = nc.dram_tensor("my1", [SHARD_PAD // 2, ROW], bf16)
    my2 = nc.dram_tensor("my2", [SHARD_PAD // 2, ROW], bf16)

    core_ids = list(range(NCORES))

    with tile.TileContext(nc) as tc:
        with (
            tc.tile_pool(name="persist", bufs=1) as pp,
            tc.tile_pool(name="gbuf", bufs=3) as gp,
            tc.tile_pool(name="work", bufs=2) as wp,
            tc.tile_pool(name="psum", bufs=2, space="PSUM") as psp,
            tc.tile_pool(name="stage", bufs=3) as sp,
        ):
            nc.gpsimd.load_library(mlp)

            # ---------- persistent tiles ----------
            idx_t = pp.tile([P, NIDX // 16], i16)
            nc.sync.dma_start(idx_t[:], idxs[:])
            pm_t = pp.tile([P, SW * 2], bf16)
            nc.sync.dma_start(pm_t[:], pmask[:])
            ident = pp.tile([P, P], f32)
            make_identity(nc, ident[:])
            zero64 = pp.tile([P, 64], f32)
            nc.vector.memset(zero64[:], 0.0)
            negone = pp.tile([P, 64], f32)
            nc.vector.memset(negone[:], -1.0)
            epsc = pp.tile([P, H1], f32)
            nc.vector.memset(epsc[:], EPS)
            ad1_all = pp.tile([P, NBLK * H1], f32)
            ad2_all = pp.tile([P, NBLK], f32)
            b1_bc = pp.tile([P, HID], f32)
            b2_bc = pp.tile([P, OUT], f32)

            small = pp.tile([1, HID], f32, tag="sm1")
            nc.sync.dma_start(small[:], b1v[:])
            nc.gpsimd.partition_broadcast(b1_bc[:], small[0:1, :])
            small2 = pp.tile([1, OUT], f32, tag="sm2")
            nc.sync.dma_start(small2[:], b2v[:])
            nc.gpsimd.partition_broadcast(b2_bc[:], small2[0:1, :])

            # ---------- W1aug = [W1 | A_s1 | A_d1] in bf16, 2 K-chunks ----------
            as1_bc = pp.tile([P, HID], f32, tag="as1b")
            sm = pp.tile([1, HID], f32, tag="sm3")
            nc.sync.dma_start(sm[:], as1[:])
            nc.gpsimd.partition_broadcast(as1_bc[:], sm[0:1, :])
            ad1_bc = pp.tile([P, HID], f32, tag="ad1b")
            sm2 = pp.tile([1, HID], f32, tag="sm4")
            nc.sync.dma_start(sm2[:], ad1[:])
            nc.gpsimd.partition_broadcast(ad1_bc[:], sm2[0:1, :])

            w1aug = []
            for k in range(2):
                w1c = wp.tile([P, HID], f32, tag="w1c")
                nc.sync.dma_start(w1c[:], w1[k * P:(k + 1) * P, :])
                aug = pp.tile([P, 80], bf16, tag=f"w1aug{k}")
                nc.vector.tensor_copy(out=aug[:, 0:HID], in_=w1c[:])
                tmp = wp.tile([P, HID], f32, tag="w1tmp")
                nc.vector.tensor_tensor(out=tmp[:], in0=w1c[:], in1=as1_bc[:], op=OP.mult)
                asr = wp.tile([P, H1], f32, tag="w1red")
                nc.vector.tensor_reduce(out=asr[:], in_=tmp[:].rearrange("p (h f) -> p h f", h=H1),
                                        op=OP.add, axis=AX.X)
                nc.vector.tensor_copy(out=aug[:, 64:72], in_=asr[:])
                nc.vector.tensor_tensor(out=tmp[:], in0=w1c[:], in1=ad1_bc[:], op=OP.mult)
                nc.vector.tensor_reduce(out=asr[:], in_=tmp[:].rearrange("p (h f) -> p h f", h=H1),
                                        op=OP.add, axis=AX.X)
                nc.vector.tensor_copy(out=aug[:, 72:80], in_=asr[:])
                w1aug.append(aug)

            # ---------- W2aug = [W2 | A_s2 | A_d2] [64, 66] bf16 ----------
            as2_bc = pp.tile([P, OUT], f32, tag="as2b")
            smb = pp.tile([1, OUT], f32, tag="sm5")
            nc.sync.dma_start(smb[:], as2[:])
            nc.gpsimd.partition_broadcast(as2_bc[:], smb[0:1, :])
            ad2_bc = pp.tile([P, OUT], f32, tag="ad2b")
            smc = pp.tile([1, OUT], f32, tag="sm6")
            nc.sync.dma_start(smc[:], ad2[:])
            nc.gpsimd.partition_broadcast(ad2_bc[:], smc[0:1, :])

            w2c = pp.tile([HID, OUT], f32, tag="w2c")
            nc.sync.dma_start(w2c[:], w2[:])
            w2aug = pp.tile([HID, 66], bf16, tag="w2aug")
            nc.vector.tensor_copy(out=w2aug[:, 0:OUT], in_=w2c[:])
            tmp2 = wp.tile([HID, OUT], f32, tag="w2tmp")
            nc.vector.tensor_tensor(out=tmp2[:], in0=w2c[:], in1=as2_bc[0:HID, :], op=OP.mult)
            red2 = wp.tile([HID, 1], f32, tag="w2red")
            nc.vector.tensor_reduce(out=red2[:], in_=tmp2[:], op=OP.add, axis=AX.X)
            nc.vector.tensor_copy(out=w2aug[:, 64:65], in_=red2[:])
            nc.vector.tensor_tensor(out=tmp2[:], in0=w2c[:], in1=ad2_bc[0:HID, :], op=OP.mult)
            nc.vector.tensor_reduce(out=red2[:], in_=tmp2[:], op=OP.add, axis=AX.X)
            nc.vector.tensor_copy(out=w2aug[:, 65:66], in_=red2[:])

            # ---------- stage 1: per block matmul -> my1 rows + a_d1 ----------
            nc.sync.dma_start(table1[0:1, :], drow[:])
            nc.sync.dma_start(table2[0:1, :], drow[:])
            for b in range(NBLK):
                ps = psp.tile([P, 80], f32, tag="s1ps")
                for k in range(2):
                    xt = sp.tile([P, P], bf16, tag="xt")
                    nc.sync.dma_start(xt[:], xT[k * P:(k + 1) * P, b * P:(b + 1) * P])
                    nc.tensor.matmul(ps[:], lhsT=xt[:], rhs=w1aug[k][:],
                                     start=(k == 0), stop=(k == 1))
                nc.vector.tensor_copy(out=ad1_all[:, b * H1:(b + 1) * H1], in_=ps[:, 72:80])
                pk_h = sp.tile([P, HID], bf16, tag="pk_h")
                nc.scalar.activation(pk_h[:], ps[:, 0:HID], AF.Copy)
                pk_as = sp.tile([P, H1], f32, tag="pk_as")
                nc.vector.tensor_copy(out=pk_as[:], in_=ps[:, 64:72])
                nc.sync.dma_start(
                    my1[b * 64:(b + 1) * 64, 0:128].rearrange("r (t f) -> r t f", t=2),
                    pk_h[:])
                nc.sync.dma_start(
                    my1[b * 64:(b + 1) * 64, 128:160].bitcast(f32).rearrange("r (t f) -> r t f", t=2),
                    pk_as[:])

            # ---------- AllGather layer-1 table ----------
            cc_sem = None
            nc.gpsimd.collective_compute(
                "AllGather", mybir.AluOpType.bypass,
                replica_groups=[core_ids],
                ins=[my1[0:SHARD // 2, :]],
                outs=[table1[1:TROWS, :]],
            )

            # ---------- edge phase helper ----------
            qq = [0]

            def edge_phase(layer, table, qbase):
                """layer 1: produces h2 per block, writes my2 rows + a_d2.
                layer 2: writes final output rows."""
                H = H1 if layer == 1 else 1
                for ci, (b0, b1, coff, ncols) in enumerate(chunks):
                    g = gp.tile([P, MAXCOLS, ROW], bf16, tag="g")
                    # split the chunk gather across all 4 SWDGE queues
                    nsub = 4 if ncols >= 8 else 1
                    o = 0
                    for si in range(nsub):
                        take = (ncols - o + (nsub - si - 1)) // (nsub - si)
                        if take == 0:
                            continue
                        nidx = take * P
                        nc.gpsimd.dma_gather(
                            g[:, o:o + take, :], table[:],
                            idx_t[:, (coff + o) * 8:(coff + o + take) * 8],
                            nidx, nidx, ROW,
                            single_packet=False, queue_num=qq[0] % 4,
                        )
                        qq[0] += 1
                        o += take
                    for b in range(b0, b1):
                        wb = int(W[b])
                        o = int(cum[b]) - coff
                        gs = g[:, o:o + wb, :]
                        # a_s gathered: [P, wb, 2, H] f32
                        if layer == 1:
                            asg = gs[:, :, 128:160].bitcast(f32).rearrange(
                                "p w (t h) -> p w t h", t=2)
                            adb = ad1_all[:, b * H1:(b + 1) * H1][:, None, None, :].to_broadcast(
                                [P, wb, 2, H])
                        else:
                            asg = gs[:, :, 128:132].bitcast(f32).rearrange(
                                "p w (t h) -> p w t h", t=2)
                            adb = ad2_all[:, b:b + 1][:, None, None, :].to_broadcast(
                                [P, wb, 2, H])
                        alpha = wp.tile([P, MAXW * 2 * H1], f32, tag="alpha")
                        al = alpha[:, 0:wb * 2 * H].rearrange("p (w t h) -> p w t h", t=2, h=H)
                        nc.vector.tensor_tensor(out=al, in0=asg, in1=adb, op=OP.add)
                        alf = alpha[:, 0:wb * 2 * H]
                        nc.scalar.activation(alf, alf, AF.Lrelu, alpha=SLOPE)
                        ex = wp.tile([P, MAXW * 2 * H1], bf16, tag="ex")
                        exv = ex[:, 0:wb * 2 * H]
                        nc.scalar.activation(exv, alf, AF.Exp)
                        # parity/pad mask
                        pmv = pm_t[:, 2 * (coff + o):2 * (coff + o) + 2 * wb].rearrange(
                            "p (w t) -> p w t", t=2)[:, :, :, None].to_broadcast([P, wb, 2, H])
                        exm = wp.tile([P, MAXW * 2 * H1], bf16, tag="exm")
                        exmv = exm[:, 0:wb * 2 * H].rearrange("p (w t h) -> p w t h", t=2, h=H)
                        nc.vector.tensor_tensor(
                            out=exmv, in0=exv.rearrange("p (w t h) -> p w t h", t=2, h=H),
                            in1=pmv, op=OP.mult)
                        # denom [P, H]
                        den = wp.tile([P, H1], f32, tag="den")
                        nc.vector.tensor_reduce(
                            out=den[:, 0:H],
                            in_=exm[:, 0:wb * 2 * H].rearrange("p (wt h) -> p h wt", h=H),
                            op=OP.add, axis=AX.X)
                        # msg product and aggregate
                        hp = gs[:, :, 0:128].rearrange("p w (t h f) -> p w t h f", t=2, h=H)
                        exb = exm[:, 0:wb * 2 * H].rearrange("p (w t h) -> p w t h", t=2, h=H)[
                            :, :, :, :, None].to_broadcast([P, wb, 2, H, 64 // H])
                        prod = wp.tile([P, MAXW * 2 * 64], bf16, tag="prod")
                        prv = prod[:, 0:wb * 2 * 64].rearrange("p (w t h f) -> p w t h f",
                                                               t=2, h=H, f=64 // H)
                        nc.vector.tensor_tensor(out=prv, in0=hp, in1=exb, op=OP.mult)
                        msum = wp.tile([P, 64], f32, tag="msum")
                        nc.vector.tensor_reduce(
                            out=msum[:],
                            in_=prod[:, 0:wb * 2 * 64].rearrange("p (wt hf) -> p hf wt", hf=64),
                            op=OP.add, axis=AX.X)
                        # divide
                        rec = wp.tile([P, H1], f32, tag="rec")
                        nc.vector.tensor_tensor(out=den[:, 0:H], in0=den[:, 0:H],
                                                in1=epsc[:, 0:H], op=OP.add)
                        nc.vector.reciprocal(out=rec[:, 0:H], in_=den[:, 0:H])
                        ob = wp.tile([P, 64], f32, tag="ob")
                        rb = rec[:, 0:H][:, :, None].to_broadcast([P, H, 64 // H])
                        nc.vector.tensor_tensor(
                            out=ob[:].rearrange("p (h f) -> p h f", h=H),
                            in0=msum[:].rearrange("p (h f) -> p h f", h=H),
                            in1=rb, op=OP.mult)
                        if layer == 1:
                            # + b1, ELU, stage-2 matmul, write my2 + a_d2
                            nc.vector.tensor_tensor(out=ob[:], in0=ob[:], in1=b1_bc[:], op=OP.add)
                            mn = wp.tile([P, 64], f32, tag="mn")
                            nc.vector.tensor_tensor(out=mn[:], in0=ob[:], in1=zero64[:], op=OP.min)
                            nc.scalar.activation(mn[:], mn[:], AF.Exp)
                            nc.vector.tensor_tensor(out=mn[:], in0=mn[:], in1=negone[:], op=OP.add)
                            h2 = wp.tile([P, 64], f32, tag="h2")
                            nc.vector.tensor_tensor(out=h2[:], in0=ob[:], in1=mn[:], op=OP.max)
                            psT = psp.tile([64, P], f32, tag="psT")
                            nc.tensor.transpose(psT[:], h2[:], ident[:])
                            h2T = sp.tile([64, P], bf16, tag="h2T")
                            nc.scalar.activation(h2T[:], psT[:], AF.Copy)
                            ps2 = psp.tile([P, 66], f32, tag="ps2")
                            nc.tensor.matmul(ps2[:], lhsT=h2T[:], rhs=w2aug[:], start=True, stop=True)
                            nc.vector.tensor_copy(out=ad2_all[:, b:b + 1], in_=ps2[:, 65:66])
                            pk2h = sp.tile([P, 64], bf16, tag="pk2h")
                            nc.scalar.activation(pk2h[:], ps2[:, 0:64], AF.Copy)
                            pk2a = sp.tile([P, 1], f32, tag="pk2a")
                            nc.vector.tensor_copy(out=pk2a[:], in_=ps2[:, 64:65])
                            nc.sync.dma_start(
                                my2[b * 64:(b + 1) * 64, 0:128].rearrange("r (t f) -> r t f", t=2),
                                pk2h[:])
                            nc.sync.dma_start(
                                my2[b * 64:(b + 1) * 64, 128:132].bitcast(f32).rearrange("r (t f) -> r t f", t=2),
                                pk2a[:])
                        else:
                            # + b2, log_softmax, write out
                            nc.vector.tensor_tensor(out=ob[:], in0=ob[:], in1=b2_bc[:], op=OP.add)
                            rmax = wp.tile([P, 1], f32, tag="rmax")
                            nc.vector.tensor_reduce(out=rmax[:], in_=ob[:], op=OP.max, axis=AX.X)
                            nrm = wp.tile([P, 1], f32, tag="nrm")
                            nc.vector.tensor_scalar_mul(out=nrm[:], in0=rmax[:], scalar1=-1.0)
                            esc = wp.tile([P, 64], f32, tag="esc")
                            rsum = wp.tile([P, 1], f32, tag="rsum")
                            nc.scalar.activation(esc[:], ob[:], AF.Exp, bias=nrm[:, 0:1],
                                                 accum_out=rsum[:, 0:1])
                            lns = wp.tile([P, 1], f32, tag="lns")
                            nc.scalar.activation(lns[:], rsum[:], AF.Ln)
                            shift = wp.tile([P, 1], f32, tag="shift")
                            nc.vector.tensor_tensor(out=shift[:], in0=rmax[:], in1=lns[:], op=OP.add)
                            fin = wp.tile([P, 64], f32, tag="fin")
                            nc.vector.tensor_tensor(
                                out=fin[:], in0=ob[:],
                                in1=shift[:, 0:1].to_broadcast([P, 64]), op=OP.subtract)
                            nc.sync.dma_start(out[b * P:(b + 1) * P, :], fin[:])

            MAXW = int(W.max())
            edge_phase(1, table1, qbase=0)
            nc.gpsimd.collective_compute(
                "AllGather", mybir.AluOpType.bypass,
                replica_groups=[core_ids],
                ins=[my2[0:SHARD // 2, :]],
                outs=[table2[1:TROWS, :]],
            )
            edge_phase(2, table2, qbase=2)

    nc.finalize()
    return nc


# ======================================================================
# entry point
# ======================================================================
def kernel(**inputs):
    x = np.asarray(inputs["x"], dtype=np.float32)
    edge_index = np.asarray(inputs["edge_index"])
    W1 = np.asarray(inputs["W1"], dtype=np.float32)
    att_src1 = np.asarray(inputs["att_src1"], dtype=np.float32)
    att_dst1 = np.asarray(inputs["att_dst1"], dtype=np.float32)
    b1 = np.asarray(inputs["b1"], dtype=np.float32)
    W2 = np.asarray(inputs["W2"], dtype=np.float32)
    att_src2 = np.asarray(inputs["att_src2"], dtype=np.float32)
    att_dst2 = np.asarray(inputs["att_dst2"], dtype=np.float32)
    b2 = np.asarray(inputs["b2"], dtype=np.float32)

    key = hash(edge_index.tobytes())
    if key not in _cache:
        plan = _build_plan(edge_index)
        nc = _build_nc(plan)
        _cache[key] = (plan, nc)
    plan, nc = _cache[key]

    # ---- stage inputs ----
    new_of_orig = plan["new_of_orig"]
    orig_of_new = plan["orig_of_new"]
    x_new = x[orig_of_new]                                   # [N, IN] new order
    dummy = np.zeros(ROW, dtype=ml_dtypes.bfloat16)
    dummy.view(np.float32)[64:80] = NEG                      # a_s slots (elems 128:160)
    dummy = dummy[None, :]

    in_maps = []
    for c in range(NCORES):
        xs = x_new[c * SHARD:(c + 1) * SHARD]
        xs = np.concatenate([xs, np.zeros((SHARD_PAD - SHARD, IN), np.float32)], axis=0)
        xT = np.ascontiguousarray(xs.T).astype(ml_dtypes.bfloat16)
        in_maps.append({
            "xT": xT.view(np.uint16),
            "idxs": plan["idx_streams"][c],
            "pmask": plan["mask_streams"][c].reshape(P, -1).view(np.uint16),
            "w1": W1, "as1": att_src1.reshape(1, -1), "ad1": att_dst1.reshape(1, -1),
            "b1v": b1.reshape(1, -1),
            "w2": W2, "as2": att_src2.reshape(1, -1), "ad2": att_dst2.reshape(1, -1),
            "b2v": b2.reshape(1, -1),
            "drow": dummy.view(np.uint16),
        })

    global _last_in_maps
    _last_in_maps = in_maps
    from concourse.bass_utils import run_bass_kernel_spmd
    res = run_bass_kernel_spmd(nc, in_maps, core_ids=list(range(NCORES)))

    full = np.zeros((N, OUT), dtype=np.float32)
    for c in range(NCORES):
        full[c * SHARD:(c + 1) * SHARD] = res.results[c]["out"][0:SHARD]
    return full[new_of_orig]


if __name__ == "__main__":
    d = np.load("/root/problem/ref_inputs.npz")
    outp = kernel(**{k: d[k] for k in d.files})
    exp = np.load("/root/problem/ref_out.npy")
    err = np.abs(outp - exp)
    print("max abs err:", err.max(), "rel:", err.max() / np.abs(exp).max())

